# revision 71
# baseline (speedup 1.0000x reference)
# Trainium2 Bass kernel: single-head causal attention (k.q^T scores, no scale)
# B=16, T=4096, D=64. Data-parallel over batch: 2 batches per NeuronCore x 8.
#
# v4 structure (per batch):
#   - bilinear merge: scores[s,t] = q_s.k_t = x_s^T (Wq^T Wk) x_t, with
#     M = Wq^T Wk precomputed on host. ONE projection z = M^T x per batch
#     (f32r matmul), z dup'd on partition halves -> z2 [128,T] bf16; the
#     score matmuls then use raw x (bf16, dup'd on halves) as the moving
#     operand, so only one projected factor exists (better numerics, half
#     the projection matmuls + drains).
#   - v projection (bf16) -> vex [128, NSB, 65] bf16 (col 64 = ones for the
#     softmax denominator)
#   - scores land in PSUM as FLOAT16 [128,1024] = ONE psum bank: 6 st
#     buffers rotate (huge pipeline slack) and the f16 scores feed the DVE
#     Schraudolph exp in 2x packed mode: int16(x*EA16+EB16) ARE the bf16
#     bits of exp(x). exp is split ACT/DVE by a running ns load balancer.
#   - causal masks on the diagonal 128-blocks on GPSIMD; PV as bf16 matmuls
#     with P as weights (N=65) accumulating into 4 interleaved PSUM regions
#   - drain: plain copy acc -> bf16 SBUF + DMA; normalization (divide by
#     the ones-column sum) happens on the HOST after gather.
import numpy as np

B, T, D = 16, 4096, 64
NCORES = 8
BPC = B // NCORES      # batches per core
TT = 512               # t-tile width
NTT = T // TT          # 8 t tiles
SB = 128               # s block
NSB = T // SB          # 32 s blocks

# Schraudolph fast-exp, int16 variant: int16(x*EA16 + EB16) are the bf16
# bits of exp(x)
EA16 = 12102203.161561485 / 65536.0        # (2^23 / ln 2) / 2^16
EB16 = float(127 * 2 ** 23 - 490000) / 65536.0

_cache = {}
MMLOG = []


SPLIT_WAITS = True     # set False for CoreSim value-testing
WARMUP = True


def _build():
    from contextlib import ExitStack
    import concourse.bass as bass
    import concourse.mybir as mybir
    import concourse.tile as tile

    f32 = mybir.dt.float32
    f32r = mybir.dt.float32r
    f16 = mybir.dt.float16
    bf16 = mybir.dt.bfloat16
    i16 = mybir.dt.int16
    EXP = mybir.ActivationFunctionType.Exp
    MULT = mybir.AluOpType.mult
    ADD = mybir.AluOpType.add

    nc = bass.Bass("TRN2", target_bir_lowering=False, debug=False,
                   enable_asserts=False)

    xT_d = nc.dram_tensor("xt", [BPC, D, T], f32r, kind="ExternalInput").ap()
    xb_d = nc.dram_tensor("xbf", [BPC, D, T], bf16, kind="ExternalInput").ap()
    wz_d = nc.dram_tensor("wz2", [D, 128], f32r, kind="ExternalInput").ap()
    wv_d = nc.dram_tensor("wvb", [D, D], bf16, kind="ExternalInput").ap()
    mk_d = nc.dram_tensor("mkb", [128, 128], bf16, kind="ExternalInput").ap()
    on_d = nc.dram_tensor("onesb", [128, NSB], bf16, kind="ExternalInput").ap()
    # raw (unnormalized) output + denominator: [..., 65*tch + h], h==64 is
    # the softmax denominator; host divides after the gather
    out_d = nc.dram_tensor("out", [BPC, NTT, SB, 4, 65], bf16,
                           kind="ExternalOutput").ap()

    # virtual-clock ACT/DVE balancer (costs mirror the TRN2 cost model):
    # each op is assigned to the engine that would FINISH it earliest given
    # a virtual PE clock driven by emitted matmul cols
    vpe = [0.0]
    load = {"act": 0.0, "dve": 0.0}

    def pick(ca, cd, lat=150.0):
        ready = vpe[0] + lat
        fa = max(load["act"], ready) + ca
        fd = max(load["dve"], ready) + cd
        if fa <= fd:
            load["act"] = fa
            return "act"
        load["dve"] = fd
        return "dve"

    # drain copies are latency-tolerant but must respect pool rotation:
    # emit them within the same slot, AFTER the slot's exp ops, so they sit
    # behind the hot exps in the ACT/DVE FIFOs
    lazy = []

    def defer_op(delay, fn):
        lazy.append(fn)

    def flush_due():
        for fn in lazy:
            fn()
        lazy.clear()

    flush_all = flush_due

    with ExitStack() as ctx:
        tc = ctx.enter_context(tile.TileContext(nc))
        consts = ctx.enter_context(tc.tile_pool(name="consts", bufs=1))
        bigp = ctx.enter_context(tc.tile_pool(name="big", bufs=2))
        ptp = ctx.enter_context(tc.tile_pool(name="pt", bufs=30))
        tmpp = ctx.enter_context(tc.tile_pool(name="tmp", bufs=30))
        stg = ctx.enter_context(tc.tile_pool(name="stg", bufs=16))
        # PSUM: st [128,1024] f32 = 2 banks x3, acc [128,260] f32 = 1 bank x2
        pst = ctx.enter_context(tc.tile_pool(name="pst", bufs=3, space="PSUM"))
        pacc = ctx.enter_context(tc.tile_pool(name="pacc", bufs=2, space="PSUM"))

        wz_sb = consts.tile([D, 128], f32r, tag="wz")
        wv_sb = consts.tile([D, D], bf16, tag="wv")
        mk_sb = consts.tile([128, 128], bf16, tag="mk")

        # PE warmup during the input-DMA wait: cheap bf16 dummy matmuls on
        # zeroed scratch keep the PE p-state ramp running until the first
        # real matmul's inputs arrive (~3.4us), without overshooting
        if WARMUP:
            dumw = consts.tile([D, 260], bf16, tag="dumw")
            nc.vector.memset(dumw[:], 0.0)
            dum_ps = pacc.tile([128, 260], f32, tag="acc", name="warm")
            for _w in range(16):
                MMLOG.append(f"warmup {_w}")
                nc.tensor.matmul(dum_ps[:, 0:260], dumw[:, 0:128],
                                 dumw[:, 0:260])

        # ---- loads: x^T f32 for the z projection, x bf16 dup'd on both
        # partition halves for scores + v projection
        xt_sb, xd_sb, z2, vex = {}, {}, {}, {}
        for b in range(BPC):
            xt_sb[b] = bigp.tile([D, T], f32r, tag="xt", name=f"xt{b}")
            xd_sb[b] = bigp.tile([128, T], bf16, tag="xd", name=f"xd{b}")
            z2[b] = bigp.tile([128, T], bf16, tag="z2", name=f"z2{b}")
            vex[b] = bigp.tile([128, NSB, 65], bf16, tag="vex", name=f"vex{b}")
        loads = [
            # startup-critical first: tiny consts + the first x chunks;
            # tiles 2,3 project at section-0 start, so xt[1024:2048] early
            ("s", wz_sb[:], wz_d[:]),
            ("s", xt_sb[0][:, 0:1024], xT_d[0][:, 0:1024]),
            ("g", xd_sb[0][0:64, 0:1024], xb_d[0][:, 0:1024]),
            ("s", xt_sb[1][:, 0:1024], xT_d[1][:, 0:1024]),
            ("g", xd_sb[0][64:128, 0:1024], xb_d[0][:, 0:1024]),
            ("g", xd_sb[1][0:64, 0:1024], xb_d[1][:, 0:1024]),
            ("g", xd_sb[1][64:128, 0:1024], xb_d[1][:, 0:1024]),
            ("s", xt_sb[0][:, 1024:2048], xT_d[0][:, 1024:2048]),
            ("s", xt_sb[1][:, 1024:2048], xT_d[1][:, 1024:2048]),
            ("s", wv_sb[:], wv_d[:]),
            ("s", mk_sb[:], mk_d[:]),
            ("s", vex[0][:, :, 64], on_d[:]),
            ("s", vex[1][:, :, 64], on_d[:]),
        ]
        for b in range(BPC):
            for h in range(2):
                loads.append(("g" if (b + h) % 2 else "s",
                              xd_sb[b][64 * h:64 * (h + 1), 1024:2560],
                              xb_d[b][:, 1024:2560]))
                loads.append(("s" if (b + h) % 2 else "g",
                              xd_sb[b][64 * h:64 * (h + 1), 2560:T],
                              xb_d[b][:, 2560:T]))
        for i in range(2, 4):
            for b in range(BPC):
                sl = slice(i * 1024, (i + 1) * 1024)
                loads.append(("s", xt_sb[b][:, sl], xT_d[b][:, sl]))
        for q, dst, src in loads:
            (nc.sync if q == "s" else nc.gpsimd).dma_start(dst, src)

        def chunk_front(b, t, c):
            """Scores + exp for chunk (b,t,c). Returns the pt source for the
            (deferred) PV stage: an AP factory offset -> [128,128]."""
            st = pst.tile([128, 1024], f32, tag="st")
            # score matmuls (row-tiled halves); trim fully-masked cols; pack
            # p1's live cols right after p0's so ONE exp covers both halves
            los = []
            for p in range(2):
                j = 2 * c + p - 4 * t
                los.append(128 * max(j, 0))
            for p in range(2):
                sblk = 2 * c + p
                lo = los[p]
                dst = lo if p == 0 else 512
                half = slice(64 * p, 64 * (p + 1))
                MMLOG.append(f"score b{b} t{t} c{c} p{p}")
                vpe[0] += (512 - lo) * 0.4167 + 5
                nc.tensor.matmul(
                    st[:, dst: dst + 512 - lo],
                    z2[b][half, sblk * SB:(sblk + 1) * SB],
                    xd_sb[b][half, t * TT + lo:(t + 1) * TT])
            span = (los[0], 512 + 512 - los[1])
            ncols = span[1] - span[0]

            def remap(o, los=los):
                # offset 512p + 128*tch -> packed position
                p_, r = divmod(o, 512)
                return r if p_ == 0 else 512 + r - los[1]

            # balance exp between ACT and Schraudolph-on-DVE by running ns
            eng = pick(ncols * 0.8333 + 185.0, ncols * 1.0417 + 125.0)
            if eng == "dve":
                tmp = tmpp.tile([128, 1024], i16, tag="tmp")
                nc.vector.tensor_scalar(
                    tmp[:, span[0]:span[1]],
                    st[:, span[0]:span[1]], EA16, EB16, MULT, ADD)
                tb = tmp[:, :].bitcast(bf16)
                src = lambda o: tb[:, remap(o):remap(o) + 128]
            else:
                pt = ptp.tile([128, 1024], bf16, tag="pt")
                nc.scalar.activation(pt[:, span[0]:span[1]],
                                     st[:, span[0]:span[1]], EXP)
                src = lambda o: pt[:, remap(o):remap(o) + 128]

            # causal mask on diagonal 128-blocks: inline on DVE for DVE
            # chunks, on the otherwise-idle GPSIMD for ACT chunks
            for p in range(2):
                j = 2 * c + p - 4 * t
                if 0 <= j <= 3:
                    tgt = src(512 * p + 128 * j)
                    if eng == "dve":
                        nc.vector.tensor_mul(tgt, tgt, mk_sb[:])
                    else:
                        nc.gpsimd.tensor_tensor(tgt, tgt, mk_sb[:],
                                                mybir.AluOpType.mult)
            return src

        def chunk_pv(b, t, c, acc, src, ctr, total):
            # PV: P as weights, V streaming (bf16, N=65). The four tch
            # sub-regions share one PSUM bank = one zero region: only the
            # bank's first matmul may carry start (it clears has_written for
            # the WHOLE 2KB region) and only the last may carry stop; in
            # between, per-element has_written gives each sub-region
            # overwrite-then-accumulate.
            for p in range(2):
                sblk = 2 * c + p
                j = sblk - 4 * t
                for tch in range(4):
                    if j > tch:
                        continue   # fully masked block
                    MMLOG.append(f"pv b{b} t{t} c{c} p{p} tch{tch}")
                    vpe[0] += 65 * 0.4167 + 5
                    nc.tensor.matmul(
                        acc[:, 65 * tch: 65 * tch + 65],
                        src(512 * p + 128 * tch),
                        vex[b][:, sblk, :],
                        start=(ctr[0] == 0),
                        stop=(ctr[0] == total - 1))
                    ctr[0] += 1

        def n_pv(t):
            total = 0
            for c in range(2 * (t + 1)):
                for p in range(2):
                    j = 2 * c + p - 4 * t
                    total += 4 - max(j, 0)
            return total

        def proj(b, t):
            # z = M^T x for the t-tile PAIR (t, t+1), dup'd on both
            # partition halves by the weight layout: two f32r matmuls into
            # one 2-bank ps slot, ONE drain copy
            ps = pst.tile([128, 1024], f32, tag="st")
            sl = slice(t * TT, (t + 2) * TT)
            for h in range(2):
                s2 = slice((t + h) * TT, (t + h + 1) * TT)
                MMLOG.append(f"zproj b{b} t{t + h}")
                vpe[0] += 512 * 0.4167 + 5
                nc.tensor.matmul(ps[:, 512 * h:512 * (h + 1)], wz_sb[:],
                                 xt_sb[b][:, s2])

            def dr(b=b, sl=sl, ps=ps):
                if pick(1024 * 0.8333 + 185.0, 1024 * 1.0417 + 125.0) == "act":
                    nc.scalar.copy(z2[b][:, sl], ps[:, 0:1024])
                else:
                    nc.vector.tensor_copy(z2[b][:, sl], ps[:, 0:1024])
            defer_op(2, dr)

        def vproj(b, g):         # v proj: 8 token-blocks of 128 per group
            psv = pst.tile([128, 8, 64], f32, tag="st")
            for k in range(8):
                tb = 8 * g + k
                MMLOG.append(f"vproj b{b} g{g} k{k}")
                vpe[0] += 64 * 0.4167 + 5
                nc.tensor.matmul(psv[:, k, :],
                                 xd_sb[b][0:64, tb * SB:(tb + 1) * SB],
                                 wv_sb[:])
            def dr(b=b, g=g, psv=psv):
                for h in range(2):
                    dst = vex[b][:, 8 * g + 4 * h:8 * g + 4 * (h + 1), 0:64]
                    srcv = psv[:, 4 * h:4 * (h + 1), :]
                    if pick(256 * 0.8333 + 185.0,
                            256 * 1.0417 + 125.0) == "act":
                        nc.scalar.copy(dst, srcv)
                    else:
                        nc.vector.tensor_copy(dst, srcv)
            defer_op(2, dr)

        # ---- fused projection + attention, the two batches interleaved as
        # independent streams, PV deferred two chunks behind exp so the PE
        # stream never head-of-line blocks on a pending exp, projection
        # pipelined one t-tile ahead so each tile's z/x is ready early.
        # Tiles processed [1..7, 0]: the tiny tile 0 lands at the kernel
        # tail, shrinking the non-overlappable end of the pipeline.
        TILE_ORDER = [1, 2, 3, 4, 5, 6, 7, 0]
        DEFER = 5

        # flat global chunk stream with section boundaries dissolved: the PV
        # for a chunk always trails its front by DEFER slots, ACROSS section
        # boundaries, so section tails never serialize the pipeline
        stream = []          # (i, t, c, k, klast)
        for i, t in enumerate(TILE_ORDER):
            order = list(range(2 * (t + 1)))
            if i > 0 and t != 0:
                order.reverse()
            for k, c in enumerate(order):
                stream.append((i, t, c, k, len(order) - 1))

        accs = {}            # (i, b) -> acc tile
        sect = {}            # i -> dict(ctr=..., total=...)

        def get_acc(i, t, b):
            if (i, b) not in accs:
                if t == 0:
                    # last tile: take its acc bank from the st pool so it
                    # doesn't wait on tile 7's drain (pacc rotation)
                    accs[(i, b)] = pst.tile([128, 260], f32, tag="st",
                                            name=f"acc{b}_{t}")
                else:
                    accs[(i, b)] = pacc.tile([128, 260], f32, tag="acc",
                                             name=f"acc{b}_{t}")
            return accs[(i, b)]

        def drain_acc(i, t):
            # plain copy (normalization happens on the host) + one batched
            # store per (b, t); emitted right after the section's last PV
            for b in range(BPC):
                on4 = stg.tile([128, 4, 65], bf16, tag="on",
                               name=f"on4_{b}_{t}")
                acc = accs.pop((i, b))
                if pick(260 * 0.8333 + 185.0,
                        260 * 1.0417 + 125.0) == "act":
                    nc.scalar.copy(on4[:, :, :], acc[:, 0:260])
                else:
                    nc.vector.tensor_copy(on4[:, :, :], acc[:, 0:260])
                (nc.sync if b == 0 else nc.gpsimd).dma_start(
                    out_d[b, t], on4[:])

        for b in range(BPC):
            proj(b, 0)     # pair (0,1)
        flush_due()

        pend = {}            # (gslot, b) -> src factory
        for g in range(len(stream) + DEFER):
            if g < len(stream):
                i, t, c, k, klast = stream[g]
                if i not in sect:
                    sect[i] = dict(ctr={b: [0] for b in range(BPC)},
                                   total=n_pv(t))
                if k == 1 and i == 0:
                    for b in range(BPC):
                        vproj(b, 0)   # blocks 0-7, needed by first PVs
                if k == 0:
                    nxt = TILE_ORDER[i + 1] if i + 1 < NTT else None
                    if nxt is not None and nxt >= 2 and nxt % 2 == 0:
                        # next tile-pair's projections at the section head:
                        # the diag chunks here leave PE + exp-engine slack
                        proj(0, nxt)
                        proj(1, nxt)
                for b in range(BPC):
                    pend[(g, b)] = chunk_front(b, t, c)
                if k == 1:
                    # batch-flush pending z/v drain copies into the
                    # diag-chunk slack, behind this slot's exps
                    flush_due()
            # deferred PV of the chunk DEFER slots back (may be from the
            # previous section)
            gp = g - DEFER
            if gp >= 0:
                ip, tp, cp, kp, klastp = stream[gp]
                s = sect[ip]
                for b in range(BPC):
                    chunk_pv(b, tp, cp, get_acc(ip, tp, b),
                             pend.pop((gp, b)), s["ctr"][b], s["total"])
                if kp == klastp:
                    drain_acc(ip, tp)
                if kp == klastp and ip <= 2:
                    # v proj group ip+1 deferred to the next diag-head
                    # slack: ready well before the section-after-next's
                    # high s-block PVs
                    for b in range(BPC):
                        defer_op(0, lambda b=b, g=ip + 1: vproj(b, g))
        flush_due()

    if SPLIT_WAITS:
        _split_matmul_waits(nc)
    return nc


def _split_matmul_waits(nc):
    """fp32/fp32r matmuls lower via an LDWEIGHTS struct with a single ISA
    wait slot; walrus refuses Matmult instructions carrying >1 sync wait.
    Move every multi-wait Matmult's waits onto a PE NoOp inserted right
    before it (engines execute their stream in order, so this is
    equivalent)."""
    import bass_rust
    import concourse.mybir as mybir
    moved = 0
    for fn in nc.m.functions:
        for bb in fn.blocks:
            il = bb.instructions
            k = 0
            while k < len(il):
                inst = il[k]
                if inst.opcode != "NoOp":
                    si = inst.sync_info
                    if si is not None and si.on_wait and len(si.on_wait) > 1:
                        waits = list(si.on_wait)
                        ups = list(si.on_update) if si.on_update else []
                        # every TPB instruction has a single ISA wait slot:
                        # one NoOp per wait, in order, before the matmul
                        for wi, w in enumerate(waits):
                            nop = mybir.InstNoOp(name=f"{inst.name}-ws{wi}",
                                                 ins=[], outs=[])
                            nop.engine = inst.engine
                            nop.sync_info = bass_rust.SyncInfo(
                                on_wait=[w], on_update=[])
                            il.insert(k, nop)
                            k += 1
                        inst.sync_info = bass_rust.SyncInfo(
                            on_wait=[], on_update=ups)
                        moved += 1
                k += 1
    return moved


def _get_nc():
    if "nc" not in _cache:
        _cache["nc"] = _build()
    return _cache["nc"]


def kernel(x, Wk, Wq, Wv):
    import ml_dtypes
    from concourse.bass_utils import run_bass_kernel_spmd

    x = np.ascontiguousarray(np.asarray(x, dtype=np.float32))
    Wk = np.asarray(Wk, dtype=np.float32)
    Wq = np.asarray(Wq, dtype=np.float32)
    Wv = np.asarray(Wv, dtype=np.float32)

    xT = np.ascontiguousarray(x.transpose(0, 2, 1))          # [B, D, T]
    xbf = np.ascontiguousarray(xT.astype(ml_dtypes.bfloat16))
    # bilinear merge: scores = x^T (Wq^T Wk) x, so only ONE on-device
    # projection z = M^T x is needed (dup'd on halves by the weight layout)
    M = (Wq.T.astype(np.float64) @ Wk.astype(np.float64)).astype(np.float32)
    wz2 = np.ascontiguousarray(np.concatenate([M, M], axis=1))   # [64,128]
    wvb = np.ascontiguousarray(Wv.T.astype(ml_dtypes.bfloat16))
    mkb = np.triu(np.ones((128, 128), dtype=np.float32)).astype(
        ml_dtypes.bfloat16)
    onesb = np.ones((128, NSB), dtype=np.float32).astype(ml_dtypes.bfloat16)

    nc = _get_nc()
    in_maps = []
    for c in range(NCORES):
        in_maps.append({
            "xt": np.ascontiguousarray(xT[BPC * c: BPC * (c + 1)]),
            "xbf": np.ascontiguousarray(xbf[BPC * c: BPC * (c + 1)]),
            "wz2": wz2, "wvb": wvb,
            "mkb": mkb, "onesb": onesb,
        })
    import os
    kw = {}
    if os.environ.get("BASS_TRACE"):
        kw = dict(trace=True, stitch_traces=False)
    res = run_bass_kernel_spmd(nc, in_maps, core_ids=list(range(NCORES)), **kw)
    _cache["last_result"] = res
    out = np.empty((B, T, D), dtype=np.float32)
    for c in range(NCORES):
        # device layout [BPC, NTT, p(128), tch(4), 65] -> [BPC, T, 65];
        # h==64 is the softmax denominator, divide on host
        o = np.asarray(res.results[c]["out"], dtype=np.float32)
        o = o.transpose(0, 1, 3, 2, 4).reshape(BPC, T, 65)
        out[BPC * c: BPC * (c + 1)] = o[:, :, 0:64] / o[:, :, 64:65]
    return out


# revision 75
# speedup vs baseline: 1.0034x; 1.0034x over previous
# Trainium2 Bass kernel: single-head causal attention (k.q^T scores, no scale)
# B=16, T=4096, D=64. Data-parallel over batch: 2 batches per NeuronCore x 8.
#
# v4 structure (per batch):
#   - bilinear merge: scores[s,t] = q_s.k_t = x_s^T (Wq^T Wk) x_t, with
#     M = Wq^T Wk precomputed on host. ONE projection z = M^T x per batch
#     (f32r matmul), z dup'd on partition halves -> z2 [128,T] bf16; the
#     score matmuls then use raw x (bf16, dup'd on halves) as the moving
#     operand, so only one projected factor exists (better numerics, half
#     the projection matmuls + drains).
#   - v projection (bf16) -> vex [128, NSB, 65] bf16 (col 64 = ones for the
#     softmax denominator)
#   - scores land in PSUM as FLOAT16 [128,1024] = ONE psum bank: 6 st
#     buffers rotate (huge pipeline slack) and the f16 scores feed the DVE
#     Schraudolph exp in 2x packed mode: int16(x*EA16+EB16) ARE the bf16
#     bits of exp(x). exp is split ACT/DVE by a running ns load balancer.
#   - causal masks on the diagonal 128-blocks on GPSIMD; PV as bf16 matmuls
#     with P as weights (N=65) accumulating into 4 interleaved PSUM regions
#   - drain: plain copy acc -> bf16 SBUF + DMA; normalization (divide by
#     the ones-column sum) happens on the HOST after gather.
import numpy as np

B, T, D = 16, 4096, 64
NCORES = 8
BPC = B // NCORES      # batches per core
TT = 512               # t-tile width
NTT = T // TT          # 8 t tiles
SB = 128               # s block
NSB = T // SB          # 32 s blocks

# Schraudolph fast-exp, int16 variant: int16(x*EA16 + EB16) are the bf16
# bits of exp(x)
EA16 = 12102203.161561485 / 65536.0        # (2^23 / ln 2) / 2^16
EB16 = float(127 * 2 ** 23 - 490000) / 65536.0

_cache = {}
MMLOG = []


SPLIT_WAITS = True     # set False for CoreSim value-testing
WARMUP = True


def _build():
    from contextlib import ExitStack
    import concourse.bass as bass
    import concourse.mybir as mybir
    import concourse.tile as tile

    f32 = mybir.dt.float32
    f32r = mybir.dt.float32r
    f16 = mybir.dt.float16
    bf16 = mybir.dt.bfloat16
    i16 = mybir.dt.int16
    EXP = mybir.ActivationFunctionType.Exp
    MULT = mybir.AluOpType.mult
    ADD = mybir.AluOpType.add

    nc = bass.Bass("TRN2", target_bir_lowering=False, debug=False,
                   enable_asserts=False)

    xT_d = nc.dram_tensor("xt", [BPC, D, T], f32r, kind="ExternalInput").ap()
    xb_d = nc.dram_tensor("xbf", [BPC, D, T], bf16, kind="ExternalInput").ap()
    wz_d = nc.dram_tensor("wz2", [D, 128], f32r, kind="ExternalInput").ap()
    wv_d = nc.dram_tensor("wvb", [D, D], bf16, kind="ExternalInput").ap()
    mk_d = nc.dram_tensor("mkb", [128, 128], bf16, kind="ExternalInput").ap()
    on_d = nc.dram_tensor("onesb", [128, NSB], bf16, kind="ExternalInput").ap()
    # raw (unnormalized) output + denominator: [..., 65*tch + h], h==64 is
    # the softmax denominator; host divides after the gather
    out_d = nc.dram_tensor("out", [BPC, NTT, SB, 4, 65], bf16,
                           kind="ExternalOutput").ap()

    # virtual-clock ACT/DVE balancer (costs mirror the TRN2 cost model):
    # each op is assigned to the engine that would FINISH it earliest given
    # a virtual PE clock driven by emitted matmul cols
    vpe = [0.0]
    load = {"act": 0.0, "dve": 0.0}

    def pick(ca, cd, lat=150.0):
        ready = vpe[0] + lat
        fa = max(load["act"], ready) + ca
        fd = max(load["dve"], ready) + cd
        if fa <= fd:
            load["act"] = fa
            return "act"
        load["dve"] = fd
        return "dve"

    # drain copies are latency-tolerant but must respect pool rotation:
    # emit them within the same slot, AFTER the slot's exp ops, so they sit
    # behind the hot exps in the ACT/DVE FIFOs
    lazy = []

    def defer_op(delay, fn):
        lazy.append(fn)

    def flush_due():
        for fn in lazy:
            fn()
        lazy.clear()

    flush_all = flush_due

    with ExitStack() as ctx:
        tc = ctx.enter_context(tile.TileContext(nc))
        consts = ctx.enter_context(tc.tile_pool(name="consts", bufs=1))
        bigp = ctx.enter_context(tc.tile_pool(name="big", bufs=2))
        ptp = ctx.enter_context(tc.tile_pool(name="pt", bufs=30))
        tmpp = ctx.enter_context(tc.tile_pool(name="tmp", bufs=30))
        stg = ctx.enter_context(tc.tile_pool(name="stg", bufs=16))
        # PSUM: st [128,1024] f32 = 2 banks x3, acc [128,260] f32 = 1 bank x2
        pst = ctx.enter_context(tc.tile_pool(name="pst", bufs=3, space="PSUM"))
        pacc = ctx.enter_context(tc.tile_pool(name="pacc", bufs=2, space="PSUM"))

        wz_sb = consts.tile([D, 128], f32r, tag="wz")
        wv_sb = consts.tile([D, D], bf16, tag="wv")
        mk_sb = consts.tile([128, 128], bf16, tag="mk")

        # PE warmup during the input-DMA wait: cheap bf16 dummy matmuls on
        # zeroed scratch keep the PE p-state ramp running until the first
        # real matmul's inputs arrive (~3.4us), without overshooting
        if WARMUP:
            dumw = consts.tile([D, 260], bf16, tag="dumw")
            nc.vector.memset(dumw[:], 0.0)
            dum_ps = pacc.tile([128, 260], f32, tag="acc", name="warm")
            for _w in range(16):
                MMLOG.append(f"warmup {_w}")
                nc.tensor.matmul(dum_ps[:, 0:260], dumw[:, 0:128],
                                 dumw[:, 0:260])

        # ---- loads: x^T f32 for the z projection, x bf16 dup'd on both
        # partition halves for scores + v projection
        xt_sb, xd_sb, z2, vex = {}, {}, {}, {}
        for b in range(BPC):
            xt_sb[b] = bigp.tile([D, T], f32r, tag="xt", name=f"xt{b}")
            xd_sb[b] = bigp.tile([128, T], bf16, tag="xd", name=f"xd{b}")
            z2[b] = bigp.tile([128, T], bf16, tag="z2", name=f"z2{b}")
            vex[b] = bigp.tile([128, NSB, 65], bf16, tag="vex", name=f"vex{b}")
        loads = [
            # startup-critical first: tiny consts + the first x chunks;
            # tiles 2,3 project at section-0 start, so xt[1024:2048] early
            ("s", wz_sb[:], wz_d[:]),
            ("s", xt_sb[0][:, 0:1024], xT_d[0][:, 0:1024]),
            ("g", xd_sb[0][0:64, 0:1024], xb_d[0][:, 0:1024]),
            ("s", xt_sb[1][:, 0:1024], xT_d[1][:, 0:1024]),
            ("g", xd_sb[0][64:128, 0:1024], xb_d[0][:, 0:1024]),
            ("g", xd_sb[1][0:64, 0:1024], xb_d[1][:, 0:1024]),
            ("g", xd_sb[1][64:128, 0:1024], xb_d[1][:, 0:1024]),
            ("s", xt_sb[0][:, 1024:2048], xT_d[0][:, 1024:2048]),
            ("s", xt_sb[1][:, 1024:2048], xT_d[1][:, 1024:2048]),
            ("s", wv_sb[:], wv_d[:]),
            ("s", mk_sb[:], mk_d[:]),
            ("s", vex[0][:, :, 64], on_d[:]),
            ("s", vex[1][:, :, 64], on_d[:]),
        ]
        for b in range(BPC):
            for h in range(2):
                loads.append(("g" if (b + h) % 2 else "s",
                              xd_sb[b][64 * h:64 * (h + 1), 1024:2560],
                              xb_d[b][:, 1024:2560]))
                loads.append(("s" if (b + h) % 2 else "g",
                              xd_sb[b][64 * h:64 * (h + 1), 2560:T],
                              xb_d[b][:, 2560:T]))
        for i in range(2, 4):
            for b in range(BPC):
                sl = slice(i * 1024, (i + 1) * 1024)
                loads.append(("s", xt_sb[b][:, sl], xT_d[b][:, sl]))
        for q, dst, src in loads:
            (nc.sync if q == "s" else nc.gpsimd).dma_start(dst, src)

        def chunk_front(b, t, c):
            """Scores + exp for chunk (b,t,c). Returns the pt source for the
            (deferred) PV stage: an AP factory offset -> [128,128]."""
            st = pst.tile([128, 1024], f32, tag="st")
            # score matmuls (row-tiled halves); trim fully-masked cols; pack
            # p1's live cols right after p0's so ONE exp covers both halves
            los = []
            for p in range(2):
                j = 2 * c + p - 4 * t
                los.append(128 * max(j, 0))
            for p in range(2):
                sblk = 2 * c + p
                lo = los[p]
                dst = lo if p == 0 else 512
                half = slice(64 * p, 64 * (p + 1))
                MMLOG.append(f"score b{b} t{t} c{c} p{p}")
                vpe[0] += (512 - lo) * 0.4167 + 5
                nc.tensor.matmul(
                    st[:, dst: dst + 512 - lo],
                    z2[b][half, sblk * SB:(sblk + 1) * SB],
                    xd_sb[b][half, t * TT + lo:(t + 1) * TT])
            span = (los[0], 512 + 512 - los[1])
            ncols = span[1] - span[0]

            def remap(o, los=los):
                # offset 512p + 128*tch -> packed position
                p_, r = divmod(o, 512)
                return r if p_ == 0 else 512 + r - los[1]

            # balance exp between ACT and Schraudolph-on-DVE by running ns
            eng = pick(ncols * 0.8333 + 185.0, ncols * 1.0417 + 125.0)
            if eng == "dve":
                tmp = tmpp.tile([128, 1024], i16, tag="tmp")
                nc.vector.tensor_scalar(
                    tmp[:, span[0]:span[1]],
                    st[:, span[0]:span[1]], EA16, EB16, MULT, ADD)
                tb = tmp[:, :].bitcast(bf16)
                src = lambda o: tb[:, remap(o):remap(o) + 128]
            else:
                pt = ptp.tile([128, 1024], bf16, tag="pt")
                nc.scalar.activation(pt[:, span[0]:span[1]],
                                     st[:, span[0]:span[1]], EXP)
                src = lambda o: pt[:, remap(o):remap(o) + 128]

            # causal mask on diagonal 128-blocks: inline on DVE for DVE
            # chunks, on the otherwise-idle GPSIMD for ACT chunks
            for p in range(2):
                j = 2 * c + p - 4 * t
                if 0 <= j <= 3:
                    tgt = src(512 * p + 128 * j)
                    if eng == "dve":
                        nc.vector.tensor_mul(tgt, tgt, mk_sb[:])
                    else:
                        nc.gpsimd.tensor_tensor(tgt, tgt, mk_sb[:],
                                                mybir.AluOpType.mult)
            return src

        def chunk_pv(b, t, c, acc, src, ctr, total):
            # PV: P as weights, V streaming (bf16, N=65). The four tch
            # sub-regions share one PSUM bank = one zero region: only the
            # bank's first matmul may carry start (it clears has_written for
            # the WHOLE 2KB region) and only the last may carry stop; in
            # between, per-element has_written gives each sub-region
            # overwrite-then-accumulate.
            for p in range(2):
                sblk = 2 * c + p
                j = sblk - 4 * t
                for tch in range(4):
                    if j > tch:
                        continue   # fully masked block
                    MMLOG.append(f"pv b{b} t{t} c{c} p{p} tch{tch}")
                    vpe[0] += 65 * 0.4167 + 5
                    nc.tensor.matmul(
                        acc[:, 65 * tch: 65 * tch + 65],
                        src(512 * p + 128 * tch),
                        vex[b][:, sblk, :],
                        start=(ctr[0] == 0),
                        stop=(ctr[0] == total - 1))
                    ctr[0] += 1

        def n_pv(t):
            total = 0
            for c in range(2 * (t + 1)):
                for p in range(2):
                    j = 2 * c + p - 4 * t
                    total += 4 - max(j, 0)
            return total

        def proj(b, t):
            # z = M^T x for the t-tile PAIR (t, t+1), dup'd on both
            # partition halves by the weight layout: two f32r matmuls into
            # one 2-bank ps slot, ONE drain copy
            ps = pst.tile([128, 1024], f32, tag="st")
            sl = slice(t * TT, (t + 2) * TT)
            for h in range(2):
                s2 = slice((t + h) * TT, (t + h + 1) * TT)
                MMLOG.append(f"zproj b{b} t{t + h}")
                vpe[0] += 512 * 0.4167 + 5
                nc.tensor.matmul(ps[:, 512 * h:512 * (h + 1)], wz_sb[:],
                                 xt_sb[b][:, s2])

            def dr(b=b, sl=sl, ps=ps):
                if pick(1024 * 0.8333 + 185.0, 1024 * 1.0417 + 125.0) == "act":
                    nc.scalar.copy(z2[b][:, sl], ps[:, 0:1024])
                else:
                    nc.vector.tensor_copy(z2[b][:, sl], ps[:, 0:1024])
            defer_op(2, dr)

        def vproj(b, g):         # v proj: 8 token-blocks of 128 per group
            psv = pst.tile([128, 8, 64], f32, tag="st")
            for k in range(8):
                tb = 8 * g + k
                MMLOG.append(f"vproj b{b} g{g} k{k}")
                vpe[0] += 64 * 0.4167 + 5
                nc.tensor.matmul(psv[:, k, :],
                                 xd_sb[b][0:64, tb * SB:(tb + 1) * SB],
                                 wv_sb[:])
            def dr(b=b, g=g, psv=psv):
                for h in range(2):
                    dst = vex[b][:, 8 * g + 4 * h:8 * g + 4 * (h + 1), 0:64]
                    srcv = psv[:, 4 * h:4 * (h + 1), :]
                    if pick(256 * 0.8333 + 185.0,
                            256 * 1.0417 + 125.0) == "act":
                        nc.scalar.copy(dst, srcv)
                    else:
                        nc.vector.tensor_copy(dst, srcv)
            defer_op(2, dr)

        # ---- fused projection + attention, the two batches interleaved as
        # independent streams, PV deferred two chunks behind exp so the PE
        # stream never head-of-line blocks on a pending exp, projection
        # pipelined one t-tile ahead so each tile's z/x is ready early.
        # Tiles processed [1..7, 0]: the tiny tile 0 lands at the kernel
        # tail, shrinking the non-overlappable end of the pipeline.
        TILE_ORDER = [1, 2, 3, 4, 5, 6, 7, 0]
        DEFER = 5

        # flat global chunk stream with section boundaries dissolved: the PV
        # for a chunk always trails its front by DEFER slots, ACROSS section
        # boundaries, so section tails never serialize the pipeline
        stream = []          # (i, t, c, k, klast)
        for i, t in enumerate(TILE_ORDER):
            order = list(range(2 * (t + 1)))
            if i > 0 and t != 0:
                order.reverse()
            for k, c in enumerate(order):
                stream.append((i, t, c, k, len(order) - 1))

        accs = {}            # (i, b) -> acc tile
        sect = {}            # i -> dict(ctr=..., total=...)

        def get_acc(i, t, b):
            if (i, b) not in accs:
                if t == 0:
                    # last tile: take its acc bank from the st pool so it
                    # doesn't wait on tile 7's drain (pacc rotation)
                    accs[(i, b)] = pst.tile([128, 260], f32, tag="st",
                                            name=f"acc{b}_{t}")
                else:
                    accs[(i, b)] = pacc.tile([128, 260], f32, tag="acc",
                                             name=f"acc{b}_{t}")
            return accs[(i, b)]

        def drain_acc(i, t):
            # plain copy (normalization happens on the host) + one batched
            # store per (b, t); emitted right after the section's last PV
            for b in range(BPC):
                on4 = stg.tile([128, 4, 65], bf16, tag="on",
                               name=f"on4_{b}_{t}")
                acc = accs.pop((i, b))
                if pick(260 * 0.8333 + 185.0,
                        260 * 1.0417 + 125.0) == "act":
                    nc.scalar.copy(on4[:, :, :], acc[:, 0:260])
                else:
                    nc.vector.tensor_copy(on4[:, :, :], acc[:, 0:260])
                (nc.sync if b == 0 else nc.gpsimd).dma_start(
                    out_d[b, t], on4[:])

        for b in range(BPC):
            proj(b, 0)     # pair (0,1)
        flush_due()

        pend = {}            # (gslot, b) -> src factory
        for g in range(len(stream) + DEFER):
            if g < len(stream):
                i, t, c, k, klast = stream[g]
                if i not in sect:
                    sect[i] = dict(ctr={b: [0] for b in range(BPC)},
                                   total=n_pv(t))
                if k == 1 and i == 0:
                    for b in range(BPC):
                        vproj(b, 0)   # blocks 0-7, needed by first PVs
                if k == 0:
                    nxt = TILE_ORDER[i + 1] if i + 1 < NTT else None
                    if nxt is not None and nxt >= 2 and nxt % 2 == 0:
                        # next tile-pair's projections at the section head:
                        # the diag chunks here leave PE + exp-engine slack
                        proj(0, nxt)
                        proj(1, nxt)
                for b in (range(BPC) if g % 3 != 0 else
                          reversed(range(BPC))):
                    pend[(g, b)] = chunk_front(b, t, c)
                if k == 1:
                    # batch-flush pending z/v drain copies into the
                    # diag-chunk slack, behind this slot's exps
                    flush_due()
            # deferred PV of the chunk DEFER slots back (may be from the
            # previous section)
            gp = g - DEFER
            if gp >= 0:
                ip, tp, cp, kp, klastp = stream[gp]
                s = sect[ip]
                for b in range(BPC):
                    chunk_pv(b, tp, cp, get_acc(ip, tp, b),
                             pend.pop((gp, b)), s["ctr"][b], s["total"])
                if kp == klastp:
                    drain_acc(ip, tp)
                if kp == klastp and ip <= 2:
                    # v proj group ip+1 deferred to the next diag-head
                    # slack: ready well before the section-after-next's
                    # high s-block PVs
                    for b in range(BPC):
                        defer_op(0, lambda b=b, g=ip + 1: vproj(b, g))
        flush_due()

    if SPLIT_WAITS:
        _split_matmul_waits(nc)
    return nc


def _split_matmul_waits(nc):
    """fp32/fp32r matmuls lower via an LDWEIGHTS struct with a single ISA
    wait slot; walrus refuses Matmult instructions carrying >1 sync wait.
    Move every multi-wait Matmult's waits onto a PE NoOp inserted right
    before it (engines execute their stream in order, so this is
    equivalent)."""
    import bass_rust
    import concourse.mybir as mybir
    moved = 0
    for fn in nc.m.functions:
        for bb in fn.blocks:
            il = bb.instructions
            k = 0
            while k < len(il):
                inst = il[k]
                if inst.opcode != "NoOp":
                    si = inst.sync_info
                    if si is not None and si.on_wait and len(si.on_wait) > 1:
                        waits = list(si.on_wait)
                        ups = list(si.on_update) if si.on_update else []
                        # every TPB instruction has a single ISA wait slot:
                        # one NoOp per wait, in order, before the matmul
                        for wi, w in enumerate(waits):
                            nop = mybir.InstNoOp(name=f"{inst.name}-ws{wi}",
                                                 ins=[], outs=[])
                            nop.engine = inst.engine
                            nop.sync_info = bass_rust.SyncInfo(
                                on_wait=[w], on_update=[])
                            il.insert(k, nop)
                            k += 1
                        inst.sync_info = bass_rust.SyncInfo(
                            on_wait=[], on_update=ups)
                        moved += 1
                k += 1
    return moved


def _get_nc():
    if "nc" not in _cache:
        _cache["nc"] = _build()
    return _cache["nc"]


def kernel(x, Wk, Wq, Wv):
    import ml_dtypes
    from concourse.bass_utils import run_bass_kernel_spmd

    x = np.ascontiguousarray(np.asarray(x, dtype=np.float32))
    Wk = np.asarray(Wk, dtype=np.float32)
    Wq = np.asarray(Wq, dtype=np.float32)
    Wv = np.asarray(Wv, dtype=np.float32)

    xT = np.ascontiguousarray(x.transpose(0, 2, 1))          # [B, D, T]
    xbf = np.ascontiguousarray(xT.astype(ml_dtypes.bfloat16))
    # bilinear merge: scores = x^T (Wq^T Wk) x, so only ONE on-device
    # projection z = M^T x is needed (dup'd on halves by the weight layout)
    M = (Wq.T.astype(np.float64) @ Wk.astype(np.float64)).astype(np.float32)
    wz2 = np.ascontiguousarray(np.concatenate([M, M], axis=1))   # [64,128]
    wvb = np.ascontiguousarray(Wv.T.astype(ml_dtypes.bfloat16))
    mkb = np.triu(np.ones((128, 128), dtype=np.float32)).astype(
        ml_dtypes.bfloat16)
    onesb = np.ones((128, NSB), dtype=np.float32).astype(ml_dtypes.bfloat16)

    nc = _get_nc()
    in_maps = []
    for c in range(NCORES):
        in_maps.append({
            "xt": np.ascontiguousarray(xT[BPC * c: BPC * (c + 1)]),
            "xbf": np.ascontiguousarray(xbf[BPC * c: BPC * (c + 1)]),
            "wz2": wz2, "wvb": wvb,
            "mkb": mkb, "onesb": onesb,
        })
    import os
    kw = {}
    if os.environ.get("BASS_TRACE"):
        kw = dict(trace=True, stitch_traces=False)
    res = run_bass_kernel_spmd(nc, in_maps, core_ids=list(range(NCORES)), **kw)
    _cache["last_result"] = res
    out = np.empty((B, T, D), dtype=np.float32)
    for c in range(NCORES):
        # device layout [BPC, NTT, p(128), tch(4), 65] -> [BPC, T, 65];
        # h==64 is the softmax denominator, divide on host
        o = np.asarray(res.results[c]["out"], dtype=np.float32)
        o = o.transpose(0, 1, 3, 2, 4).reshape(BPC, T, 65)
        out[BPC * c: BPC * (c + 1)] = o[:, :, 0:64] / o[:, :, 64:65]
    return out
